# revision 27
# baseline (speedup 1.0000x reference)
"""Trainium2 Bass kernel for windowed mean-pooling (segment_reduce).

Computes, for each (batch b, window w):
    out[b, w, :] = mean over t in [begins[b,w], ends'[b,w]) of features[b, t, :]
where ends' = clip(ends, begins, begins + 8) (the reference gathers at most
MAX_WINDOW=8 tokens) and empty windows produce 0 (count clamped to >= 1).

Strategy (data-parallel over batch, one sample per NeuronCore).  The kernel
is DMA-wire-bound with the TensorEngine close behind, so every change
attacks bytes-on-the-wire or PE columns:
  - fp8 DoubleRow: features ship as TWO fp8e4m3 planes A + R with
    A = fp8(x), R = fp8(x - A) (combined rel err ~8e-4); each masked matmul
    out_blk = S^T @ F contracts both planes (K=256) in the time fp16
    contracts 128, at the same 2 bytes/element of HBM traffic.
  - Token compaction: tokens covered by no window (~10%) are dropped
    host-side per core (windows index contiguous runs, all of whose tokens
    are covered, so begins/ends just remap), cutting feature bytes and
    K-tiles from 32 to ~30.
  - Window permutation: windows are reassigned to ~17 partially-filled
    blocks with data-adaptive K-tile ranges (pad slots get null masks),
    shrinking Sigma = total (block, K-tile) matmul count from 62 (sorted
    blocks, 8-core union) to ~46, near the combinatorial floor.
  - The 0/1 masks are built in fp8 (exact) by the VectorEngine from
    begin/end rows that the HOST pre-replicates across the 128 partitions
    (a contiguous 1.1 MB input: cheaper than burning PE/GPSIMD cycles on
    an on-chip broadcast, which measurably stalls the feature stream).
  - Output is written fp16 (host upcasts; ~3e-4 rel err), halving DMA-out
    bytes; only really-filled block rows are evacuated/written.  PSUM
    scaling by 1/count alternates ScalarEngine / VectorEngine so the
    end-of-stream evacuation chain is half as long.
  - Slab layout in SBUF: token t on partition (t % 128), K-tile (t // 128),
    planes [A; R] contiguous per K-tile.
  - DMA assignment: features via GPSIMD SWDGE only (sharing that engine or
    spreading chunks over the HWDGE queues starves the stream), metadata
    on ACT, ioiv + outputs on the SP HWDGE ring.
"""

import os
import sys

import numpy as np

for _p in ("/opt/trn_rl_repo", "/root/.axon_site/_ro/trn_rl_repo"):
    if os.path.isdir(_p) and _p not in sys.path:
        sys.path.insert(0, _p)

from concourse import bacc, mybir  # noqa: E402
import concourse.tile as tile  # noqa: E402
from concourse.bass_utils import run_bass_kernel_spmd  # noqa: E402

B, T, D, W = 8, 4096, 768, 2048
MAXWIN = 8
P = 128
LSPAN = 3  # tiles per block range in the adaptive assignment
BCH = 512  # windows per metadata DMA chunk
F32 = mybir.dt.float32
FP16 = mybir.dt.float16
FP8 = mybir.dt.float8e4
DR = mybir.MatmulPerfMode.DoubleRow


def _fchunks(nkt):
    """(K-tiles, queue) per feature DMA chunk: small first (PE starts
    early), small last (short end-of-stream drain).  Chunks round-robin
    over the three DMA queues: descriptor generation is serialized per
    engine (~1.9 us/chunk on the SWDGE path) and pacing the stream off a
    single queue caps it at ~280 GB/s."""
    sizes = [1, 1, 2]
    rem = nkt - 8
    while rem > 0:
        s = min(6, rem)
        sizes.append(s)
        rem -= s
    sizes += [2, 1, 1]
    assert sum(sizes) == nkt
    return sizes


def _assign(bp, ep, nkt):
    """Adaptive shared block ranges + per-core window permutation on
    compacted token coordinates.

    Returns (ranges, perms): ranges = [[klo_j, khi_j]], perms[c][j] = list
    of 128 window ids (-1 = pad).  Every window's span tiles fit its
    block's range; blocks may be partially filled.
    """
    nb_cores = bp.shape[0]
    UV = []
    for c in range(nb_cores):
        u = bp[c] // P
        v = np.maximum(ep[c] - 1, bp[c]) // P
        UV.append((u, v))
    unass = [set(range(W)) for _ in range(nb_cores)]
    ranges = []
    perms = [[] for _ in range(nb_cores)]
    s_prev = None
    while any(unass):
        mins = [
            min(UV[c][0][w] for w in unass[c])
            for c in range(nb_cores)
            if unass[c]
        ]
        s = min(mins)
        if s_prev is not None:
            s = max(min(s, s_prev + (LSPAN - 1)), s_prev)
        hi = min(s + LSPAN, nkt)
        ranges.append([s, hi])
        for c in range(nb_cores):
            u, v = UV[c]
            elig = [w for w in unass[c] if u[w] >= s and v[w] < hi]
            elig.sort(key=lambda w: (v[w], u[w]))
            take = elig[:P]
            unass[c] -= set(take)
            perms[c].append(take + [-1] * (P - len(take)))
        s_prev = s
        assert len(ranges) <= 24, "assignment runaway"
    # tighten each block's range to the span actually used (any core)
    for j in range(len(ranges)):
        lo, hi = nkt, 0
        for c in range(nb_cores):
            u, v = UV[c]
            ws = [w for w in perms[c][j] if w >= 0]
            if ws:
                lo = min(lo, min(u[w] for w in ws))
                hi = max(hi, max(v[w] for w in ws) + 1)
        if lo < hi:
            ranges[j] = [lo, hi]
    return ranges, perms


def _build_program(ranges, fills, nkt):
    """Build the SPMD Bass program for the given per-block K-tile ranges.

    fills[j] = max (over cores) number of real windows in block j; PSUM
    evacuation + output DMA are sliced to that partition range.
    """
    nb = len(ranges)
    Wp = nb * P
    nc = bacc.Bacc(None)

    fhi_d = nc.declare_dram_parameter("fhi", [P, nkt, 2, D], FP8, isOutput=False)
    meta = nc.declare_dram_parameter("meta", [1, 2, Wp], FP16, isOutput=False)
    ioiv = nc.declare_dram_parameter("ioiv", [P, 64], F32, isOutput=False)
    out_d = nc.declare_dram_parameter("out", [Wp, D], FP16, isOutput=True)

    fhi_r = fhi_d[:]
    out_r = out_d[:].rearrange("(n p) d -> p n d", p=P)

    # For each K-tile, the contiguous span of blocks that consume it.
    strip_rng = {}
    for k in range(nkt):
        blks = [j for j in range(nb) if ranges[j][0] <= k < ranges[j][1]]
        if blks:
            strip_rng[k] = (min(blks), max(blks) + 1)

    with tile.TileContext(nc) as tc:
        with (
            tc.tile_pool(name="metap", bufs=1) as meta_pool,
            tc.tile_pool(name="fslab", bufs=1) as f_pool,
            tc.tile_pool(name="m2p", bufs=4) as m2_pool,
            tc.tile_pool(name="maskp", bufs=12) as mask_pool,
            tc.tile_pool(name="outp", bufs=8) as out_pool,
            tc.tile_pool(name="psum", bufs=4, space="PSUM") as psum_pool,
        ):
            # iota [P, :nkt] (iota[p, k] = 128k + p - 2048), 1/count
            # [P, nkt:nkt+nb], padded to [P, 64].
            ioiv_sb = meta_pool.tile([P, 64], F32)
            nc.sync.dma_start(out=ioiv_sb[:], in_=ioiv[:])
            io_sb = ioiv_sb[:, 0:nkt]
            iv_sb = ioiv_sb[:, nkt : nkt + nb]

            # begins/ends: ONE fp16 row (shifted -2048) broadcast across
            # the 128 partitions by stride-0 DRAM->SBUF DMAs on the ACT
            # HWDGE ring, chunked so early mask builds start first.
            be_sb = meta_pool.tile([P, 2, Wp], FP16)
            for c0 in range(0, Wp, BCH):
                c1 = min(c0 + BCH, Wp)
                nc.scalar.dma_start(
                    out=be_sb[:, :, c0:c1],
                    in_=meta[:, :, c0:c1].broadcast_to((P, 2, c1 - c0)),
                )

            # Feature slab chunks (fp8 planes A,R per K-tile).
            fhi_tiles = []
            k2chunk = []
            k0 = 0
            for j, sz in enumerate(_fchunks(nkt)):
                fh = f_pool.tile([P, sz, 2, D], FP8, name=f"fh{j}", tag=f"fh{j}")
                nc.gpsimd.dma_start(out=fh[:], in_=fhi_r[:, k0 : k0 + sz, :, :])
                fhi_tiles.append(fh)
                for s in range(sz):
                    k2chunk.append((j, s))
                k0 += sz
            assert k0 == nkt

            # Per-K-tile mask strips over the span of blocks that use them,
            # in [token, window] layout: mask[p, w] = (b[w] <= t) * (e[w] > t)
            # with t = 128k + p.  fp8 (0/1 exact) for the DoubleRow matmul.
            masks = {}
            for k in sorted(strip_rng):
                blo, bhi = strip_rng[k]
                wlo, whi = blo * P, bhi * P
                wn = whi - wlo
                m2 = m2_pool.tile([P, wn], FP16, name=f"m2_{k}", tag="m2")
                msk = mask_pool.tile([P, wn], FP8, name=f"mask_{k}", tag="mask")
                nc.vector.tensor_scalar(
                    m2[:], be_sb[:, 1, wlo:whi], io_sb[:, k : k + 1], None,
                    mybir.AluOpType.is_gt,
                )
                nc.vector.scalar_tensor_tensor(
                    msk[:], be_sb[:, 0, wlo:whi], io_sb[:, k : k + 1], m2[:],
                    mybir.AluOpType.is_le, mybir.AluOpType.mult,
                )
                masks[k] = (msk, blo)

            for j in range(nb):
                klo, khi = ranges[j]
                ps = psum_pool.tile([P, D], F32, name=f"ps{j}", tag="ps")
                for k in range(klo, khi):
                    msk, blo = masks[k]
                    # Same 0/1 mask feeds both DoubleRow planes via a
                    # stride-0 middle AP dim.
                    lh = (
                        msk[:, (j - blo) * P : (j - blo + 1) * P]
                        .unsqueeze(1)
                        .broadcast_to((P, 2, P))
                    )
                    cj, cs = k2chunk[k]
                    rh = fhi_tiles[cj]
                    first = k == klo
                    last = k == khi - 1
                    for n0, nn in ((0, 512), (512, 256)):
                        nc.tensor.matmul(
                            ps[:, n0 : n0 + nn],
                            lh,
                            rh[:, cs, :, n0 : n0 + nn],
                            start=first,
                            stop=last,
                            perf_mode=DR,
                        )
                mf = fills[j]
                os = out_pool.tile([P, D], FP16, name=f"os{j}", tag="os")
                # The serial ACT evac chain (17 x ~1 us) is the pipeline
                # tail.  Late blocks evacuate on the DVE instead, which is
                # idle once mask building ends; early blocks stay on ACT so
                # the in-order DVE queue never delays a mask build.
                if j < nb - 7:
                    nc.scalar.mul(
                        out=os[0:mf, :], in_=ps[0:mf, :],
                        mul=iv_sb[0:mf, j : j + 1],
                    )
                else:
                    nc.vector.tensor_scalar_mul(
                        os[0:mf, :], ps[0:mf, :], iv_sb[0:mf, j : j + 1]
                    )
                # Outputs on the SP ring; only real rows are written.
                nc.sync.dma_start(out=out_r[0:mf, j, :], in_=os[0:mf, :])

    nc.finalize()
    return nc


def _prepare(features, begins, ends):
    feats = np.asarray(features, dtype=np.float32)
    assert feats.shape == (B, T, D), feats.shape
    b = np.clip(np.asarray(begins).astype(np.int64), 0, T - 1)
    e = np.asarray(ends).astype(np.int64)
    # Reference gathers at most MAXWIN tokens starting at b; empty -> count 1.
    e_eff = np.clip(e, b, np.minimum(b + MAXWIN, T))
    counts = np.maximum(e_eff - b, 1).astype(np.float32)
    inv = (1.0 / counts).astype(np.float32)

    # Token compaction: drop tokens covered by no window (per core).  Every
    # window's run stays contiguous (all its tokens are covered), so
    # begins/ends just remap through the position table.
    f8 = mybir.dt.np(FP8)
    covs, poss, Tps = [], [], []
    for c in range(B):
        dif = np.zeros(T + 1, np.int64)
        np.add.at(dif, b[c], 1)
        np.add.at(dif, e_eff[c], -1)
        cov = np.cumsum(dif)[:T] > 0
        covs.append(cov)
        poss.append(np.cumsum(cov) - 1)
        Tps.append(int(cov.sum()))
    nkt = max(int(np.ceil(tp / P)) for tp in Tps)
    Tp_tok = nkt * P

    bp = np.zeros((B, W), np.int64)
    ep = np.zeros((B, W), np.int64)
    his = []
    for c in range(B):
        nonempty = e_eff[c] > b[c]
        bp[c][nonempty] = poss[c][b[c][nonempty]]
        ep[c] = bp[c] + (e_eff[c] - b[c])
        fc = np.zeros((Tp_tok, D), np.float32)
        fc[: Tps[c]] = feats[c][covs[c]]
        A = fc.astype(f8)
        R = (fc - A.astype(np.float32)).astype(f8)
        his.append(
            np.ascontiguousarray(
                np.stack(
                    [A.reshape(nkt, P, D), R.reshape(nkt, P, D)], axis=2
                ).transpose(1, 0, 2, 3)
            )
        )  # [P, nkt, 2, D]

    ranges, perms = _assign(bp, ep, nkt)
    nb = len(ranges)
    Wp = nb * P

    # per-block max fill across cores (shared program slices to this)
    fills = [
        max(sum(1 for w in perms[c][j] if w >= 0) for c in range(B))
        for j in range(nb)
    ]
    fills = [max(f, 1) for f in fills]

    iota = (
        np.arange(nkt)[None, :] * P + np.arange(P)[:, None] - 2048
    ).astype(np.float32)
    in_maps = []
    idx_maps = []
    for c in range(B):
        idx = np.array(
            [w for blk in perms[c] for w in blk], dtype=np.int64
        )  # [Wp], -1 = pad
        used = idx >= 0
        bpr = np.full(Wp, 2047 + 2048, np.int64)  # pad: begin beyond tokens
        epr = np.zeros(Wp, np.int64)  # pad: end before any token
        bpr[used] = bp[c][idx[used]]
        epr[used] = ep[c][idx[used]]
        metac = np.ascontiguousarray(
            (np.stack([bpr, epr]) - 2048).astype(np.float16).reshape(1, 2, Wp)
        )
        invc = np.ones(Wp, np.float32)
        invc[used] = inv[c][idx[used]]
        ioiv_c = np.zeros((P, 64), np.float32)
        ioiv_c[:, 0:nkt] = iota
        ioiv_c[:, nkt : nkt + nb] = invc.reshape(nb, P).T
        in_maps.append({"fhi": his[c], "meta": metac, "ioiv": ioiv_c})
        idx_maps.append(idx)
    return ranges, fills, nkt, in_maps, idx_maps


def run(features, begins, ends, trace=False):
    """Build + run on 8 NeuronCores; returns (output, BassKernelResults)."""
    ranges, fills, nkt, in_maps, idx_maps = _prepare(features, begins, ends)
    nc = _build_program(ranges, fills, nkt)
    res = run_bass_kernel_spmd(nc, in_maps, list(range(B)), trace=trace)
    out = np.zeros((B, W, D), np.float32)
    for c in range(B):
        idx = idx_maps[c]
        used = idx >= 0
        dev = res.results[c]["out"].astype(np.float32)
        out[c, idx[used]] = dev[used]
    return out, res


def kernel(features, begins, ends):
    out, _ = run(features, begins, ends, trace=False)
    return out


# revision 28
# speedup vs baseline: 1.0045x; 1.0045x over previous
"""Trainium2 Bass kernel for windowed mean-pooling (segment_reduce).

Computes, for each (batch b, window w):
    out[b, w, :] = mean over t in [begins[b,w], ends'[b,w]) of features[b, t, :]
where ends' = clip(ends, begins, begins + 8) (the reference gathers at most
MAX_WINDOW=8 tokens) and empty windows produce 0 (count clamped to >= 1).

Strategy (data-parallel over batch, one sample per NeuronCore).  The kernel
is DMA-wire-bound with the TensorEngine close behind, so every change
attacks bytes-on-the-wire or PE columns:
  - fp8 DoubleRow: features ship as TWO fp8e4m3 planes A + R with
    A = fp8(x), R = fp8(x - A) (combined rel err ~8e-4); each masked matmul
    out_blk = S^T @ F contracts both planes (K=256) in the time fp16
    contracts 128, at the same 2 bytes/element of HBM traffic.
  - Token compaction: tokens covered by no window (~10%) are dropped
    host-side per core (windows index contiguous runs, all of whose tokens
    are covered, so begins/ends just remap), cutting feature bytes and
    K-tiles from 32 to ~30.
  - Window permutation: windows are reassigned to ~17 partially-filled
    blocks with data-adaptive K-tile ranges (pad slots get null masks),
    shrinking Sigma = total (block, K-tile) matmul count from 62 (sorted
    blocks, 8-core union) to ~46, near the combinatorial floor.
  - The 0/1 masks are built in fp8 (exact) by the VectorEngine from
    begin/end rows that the HOST pre-replicates across the 128 partitions
    (a contiguous 1.1 MB input: cheaper than burning PE/GPSIMD cycles on
    an on-chip broadcast, which measurably stalls the feature stream).
  - Output is written fp16 (host upcasts; ~3e-4 rel err), halving DMA-out
    bytes; only really-filled block rows are evacuated/written.  PSUM
    scaling by 1/count alternates ScalarEngine / VectorEngine so the
    end-of-stream evacuation chain is half as long.
  - Slab layout in SBUF: token t on partition (t % 128), K-tile (t // 128),
    planes [A; R] contiguous per K-tile.
  - DMA assignment: features via GPSIMD SWDGE only (sharing that engine or
    spreading chunks over the HWDGE queues starves the stream), metadata
    on ACT, ioiv + outputs on the SP HWDGE ring.
"""

import os
import sys

import numpy as np

for _p in ("/opt/trn_rl_repo", "/root/.axon_site/_ro/trn_rl_repo"):
    if os.path.isdir(_p) and _p not in sys.path:
        sys.path.insert(0, _p)

from concourse import bacc, mybir  # noqa: E402
import concourse.tile as tile  # noqa: E402
from concourse.bass_utils import run_bass_kernel_spmd  # noqa: E402

B, T, D, W = 8, 4096, 768, 2048
MAXWIN = 8
P = 128
LSPAN = 3  # tiles per block range in the adaptive assignment
BCH = 512  # windows per metadata DMA chunk
F32 = mybir.dt.float32
FP16 = mybir.dt.float16
FP8 = mybir.dt.float8e4
DR = mybir.MatmulPerfMode.DoubleRow


def _fchunks(nkt):
    """(K-tiles, queue) per feature DMA chunk: small first (PE starts
    early), small last (short end-of-stream drain).  Chunks round-robin
    over the three DMA queues: descriptor generation is serialized per
    engine (~1.9 us/chunk on the SWDGE path) and pacing the stream off a
    single queue caps it at ~280 GB/s."""
    sizes = [1, 1, 2]
    rem = nkt - 8
    while rem > 0:
        s = min(4, rem)
        sizes.append(s)
        rem -= s
    sizes += [2, 1, 1]
    assert sum(sizes) == nkt
    return sizes


def _assign(bp, ep, nkt):
    """Adaptive shared block ranges + per-core window permutation on
    compacted token coordinates.

    Returns (ranges, perms): ranges = [[klo_j, khi_j]], perms[c][j] = list
    of 128 window ids (-1 = pad).  Every window's span tiles fit its
    block's range; blocks may be partially filled.
    """
    nb_cores = bp.shape[0]
    UV = []
    for c in range(nb_cores):
        u = bp[c] // P
        v = np.maximum(ep[c] - 1, bp[c]) // P
        UV.append((u, v))
    unass = [set(range(W)) for _ in range(nb_cores)]
    ranges = []
    perms = [[] for _ in range(nb_cores)]
    s_prev = None
    while any(unass):
        mins = [
            min(UV[c][0][w] for w in unass[c])
            for c in range(nb_cores)
            if unass[c]
        ]
        s = min(mins)
        if s_prev is not None:
            s = max(min(s, s_prev + (LSPAN - 1)), s_prev)
        hi = min(s + LSPAN, nkt)
        ranges.append([s, hi])
        for c in range(nb_cores):
            u, v = UV[c]
            elig = [w for w in unass[c] if u[w] >= s and v[w] < hi]
            elig.sort(key=lambda w: (v[w], u[w]))
            take = elig[:P]
            unass[c] -= set(take)
            perms[c].append(take + [-1] * (P - len(take)))
        s_prev = s
        assert len(ranges) <= 24, "assignment runaway"
    # tighten each block's range to the span actually used (any core)
    for j in range(len(ranges)):
        lo, hi = nkt, 0
        for c in range(nb_cores):
            u, v = UV[c]
            ws = [w for w in perms[c][j] if w >= 0]
            if ws:
                lo = min(lo, min(u[w] for w in ws))
                hi = max(hi, max(v[w] for w in ws) + 1)
        if lo < hi:
            ranges[j] = [lo, hi]
    return ranges, perms


def _build_program(ranges, fills, nkt):
    """Build the SPMD Bass program for the given per-block K-tile ranges.

    fills[j] = max (over cores) number of real windows in block j; PSUM
    evacuation + output DMA are sliced to that partition range.
    """
    nb = len(ranges)
    Wp = nb * P
    nc = bacc.Bacc(None)

    fhi_d = nc.declare_dram_parameter("fhi", [P, nkt, 2, D], FP8, isOutput=False)
    meta = nc.declare_dram_parameter("meta", [1, 2, Wp], FP16, isOutput=False)
    ioiv = nc.declare_dram_parameter("ioiv", [P, 64], F32, isOutput=False)
    out_d = nc.declare_dram_parameter("out", [Wp, D], FP16, isOutput=True)

    fhi_r = fhi_d[:]
    out_r = out_d[:].rearrange("(n p) d -> p n d", p=P)

    # For each K-tile, the contiguous span of blocks that consume it.
    strip_rng = {}
    for k in range(nkt):
        blks = [j for j in range(nb) if ranges[j][0] <= k < ranges[j][1]]
        if blks:
            strip_rng[k] = (min(blks), max(blks) + 1)

    with tile.TileContext(nc) as tc:
        with (
            tc.tile_pool(name="metap", bufs=1) as meta_pool,
            tc.tile_pool(name="fslab", bufs=1) as f_pool,
            tc.tile_pool(name="m2p", bufs=4) as m2_pool,
            tc.tile_pool(name="maskp", bufs=12) as mask_pool,
            tc.tile_pool(name="outp", bufs=8) as out_pool,
            tc.tile_pool(name="psum", bufs=4, space="PSUM") as psum_pool,
        ):
            # iota [P, :nkt] (iota[p, k] = 128k + p - 2048), 1/count
            # [P, nkt:nkt+nb], padded to [P, 64].
            ioiv_sb = meta_pool.tile([P, 64], F32)
            nc.sync.dma_start(out=ioiv_sb[:], in_=ioiv[:])
            io_sb = ioiv_sb[:, 0:nkt]
            iv_sb = ioiv_sb[:, nkt : nkt + nb]

            # begins/ends: ONE fp16 row (shifted -2048) broadcast across
            # the 128 partitions by stride-0 DRAM->SBUF DMAs on the ACT
            # HWDGE ring, chunked so early mask builds start first.
            be_sb = meta_pool.tile([P, 2, Wp], FP16)
            for c0 in range(0, Wp, BCH):
                c1 = min(c0 + BCH, Wp)
                nc.scalar.dma_start(
                    out=be_sb[:, :, c0:c1],
                    in_=meta[:, :, c0:c1].broadcast_to((P, 2, c1 - c0)),
                )

            # Feature slab chunks (fp8 planes A,R per K-tile).
            fhi_tiles = []
            k2chunk = []
            k0 = 0
            for j, sz in enumerate(_fchunks(nkt)):
                fh = f_pool.tile([P, sz, 2, D], FP8, name=f"fh{j}", tag=f"fh{j}")
                nc.gpsimd.dma_start(out=fh[:], in_=fhi_r[:, k0 : k0 + sz, :, :])
                fhi_tiles.append(fh)
                for s in range(sz):
                    k2chunk.append((j, s))
                k0 += sz
            assert k0 == nkt

            # Per-K-tile mask strips over the span of blocks that use them,
            # in [token, window] layout: mask[p, w] = (b[w] <= t) * (e[w] > t)
            # with t = 128k + p.  fp8 (0/1 exact) for the DoubleRow matmul.
            masks = {}
            for k in sorted(strip_rng):
                blo, bhi = strip_rng[k]
                wlo, whi = blo * P, bhi * P
                wn = whi - wlo
                m2 = m2_pool.tile([P, wn], FP16, name=f"m2_{k}", tag="m2")
                msk = mask_pool.tile([P, wn], FP8, name=f"mask_{k}", tag="mask")
                nc.vector.tensor_scalar(
                    m2[:], be_sb[:, 1, wlo:whi], io_sb[:, k : k + 1], None,
                    mybir.AluOpType.is_gt,
                )
                nc.vector.scalar_tensor_tensor(
                    msk[:], be_sb[:, 0, wlo:whi], io_sb[:, k : k + 1], m2[:],
                    mybir.AluOpType.is_le, mybir.AluOpType.mult,
                )
                masks[k] = (msk, blo)

            for j in range(nb):
                klo, khi = ranges[j]
                ps = psum_pool.tile([P, D], F32, name=f"ps{j}", tag="ps")
                for k in range(klo, khi):
                    msk, blo = masks[k]
                    # Same 0/1 mask feeds both DoubleRow planes via a
                    # stride-0 middle AP dim.
                    lh = (
                        msk[:, (j - blo) * P : (j - blo + 1) * P]
                        .unsqueeze(1)
                        .broadcast_to((P, 2, P))
                    )
                    cj, cs = k2chunk[k]
                    rh = fhi_tiles[cj]
                    first = k == klo
                    last = k == khi - 1
                    for n0, nn in ((0, 512), (512, 256)):
                        nc.tensor.matmul(
                            ps[:, n0 : n0 + nn],
                            lh,
                            rh[:, cs, :, n0 : n0 + nn],
                            start=first,
                            stop=last,
                            perf_mode=DR,
                        )
                mf = fills[j]
                os = out_pool.tile([P, D], FP16, name=f"os{j}", tag="os")
                # The serial ACT evac chain (17 x ~1 us) is the pipeline
                # tail.  Late blocks evacuate on the DVE instead, which is
                # idle once mask building ends; early blocks stay on ACT so
                # the in-order DVE queue never delays a mask build.
                if j < nb - 7:
                    nc.scalar.mul(
                        out=os[0:mf, :], in_=ps[0:mf, :],
                        mul=iv_sb[0:mf, j : j + 1],
                    )
                else:
                    nc.vector.tensor_scalar_mul(
                        os[0:mf, :], ps[0:mf, :], iv_sb[0:mf, j : j + 1]
                    )
                # Outputs alternate between the SP and ACT HWDGE queues so
                # the trailing blocks' writes drain two rings in parallel;
                # only real rows are written.
                oeng = nc.sync if j % 2 == 0 else nc.scalar
                oeng.dma_start(out=out_r[0:mf, j, :], in_=os[0:mf, :])

    nc.finalize()
    return nc


def _prepare(features, begins, ends):
    feats = np.asarray(features, dtype=np.float32)
    assert feats.shape == (B, T, D), feats.shape
    b = np.clip(np.asarray(begins).astype(np.int64), 0, T - 1)
    e = np.asarray(ends).astype(np.int64)
    # Reference gathers at most MAXWIN tokens starting at b; empty -> count 1.
    e_eff = np.clip(e, b, np.minimum(b + MAXWIN, T))
    counts = np.maximum(e_eff - b, 1).astype(np.float32)
    inv = (1.0 / counts).astype(np.float32)

    # Token compaction: drop tokens covered by no window (per core).  Every
    # window's run stays contiguous (all its tokens are covered), so
    # begins/ends just remap through the position table.
    f8 = mybir.dt.np(FP8)
    covs, poss, Tps = [], [], []
    for c in range(B):
        dif = np.zeros(T + 1, np.int64)
        np.add.at(dif, b[c], 1)
        np.add.at(dif, e_eff[c], -1)
        cov = np.cumsum(dif)[:T] > 0
        covs.append(cov)
        poss.append(np.cumsum(cov) - 1)
        Tps.append(int(cov.sum()))
    nkt = max(int(np.ceil(tp / P)) for tp in Tps)
    Tp_tok = nkt * P

    bp = np.zeros((B, W), np.int64)
    ep = np.zeros((B, W), np.int64)
    his = []
    for c in range(B):
        nonempty = e_eff[c] > b[c]
        bp[c][nonempty] = poss[c][b[c][nonempty]]
        ep[c] = bp[c] + (e_eff[c] - b[c])
        fc = np.zeros((Tp_tok, D), np.float32)
        fc[: Tps[c]] = feats[c][covs[c]]
        A = fc.astype(f8)
        R = (fc - A.astype(np.float32)).astype(f8)
        his.append(
            np.ascontiguousarray(
                np.stack(
                    [A.reshape(nkt, P, D), R.reshape(nkt, P, D)], axis=2
                ).transpose(1, 0, 2, 3)
            )
        )  # [P, nkt, 2, D]

    ranges, perms = _assign(bp, ep, nkt)
    nb = len(ranges)
    Wp = nb * P

    # per-block max fill across cores (shared program slices to this)
    fills = [
        max(sum(1 for w in perms[c][j] if w >= 0) for c in range(B))
        for j in range(nb)
    ]
    fills = [max(f, 1) for f in fills]

    iota = (
        np.arange(nkt)[None, :] * P + np.arange(P)[:, None] - 2048
    ).astype(np.float32)
    in_maps = []
    idx_maps = []
    for c in range(B):
        idx = np.array(
            [w for blk in perms[c] for w in blk], dtype=np.int64
        )  # [Wp], -1 = pad
        used = idx >= 0
        bpr = np.full(Wp, 2047 + 2048, np.int64)  # pad: begin beyond tokens
        epr = np.zeros(Wp, np.int64)  # pad: end before any token
        bpr[used] = bp[c][idx[used]]
        epr[used] = ep[c][idx[used]]
        metac = np.ascontiguousarray(
            (np.stack([bpr, epr]) - 2048).astype(np.float16).reshape(1, 2, Wp)
        )
        invc = np.ones(Wp, np.float32)
        invc[used] = inv[c][idx[used]]
        ioiv_c = np.zeros((P, 64), np.float32)
        ioiv_c[:, 0:nkt] = iota
        ioiv_c[:, nkt : nkt + nb] = invc.reshape(nb, P).T
        in_maps.append({"fhi": his[c], "meta": metac, "ioiv": ioiv_c})
        idx_maps.append(idx)
    return ranges, fills, nkt, in_maps, idx_maps


def run(features, begins, ends, trace=False):
    """Build + run on 8 NeuronCores; returns (output, BassKernelResults)."""
    ranges, fills, nkt, in_maps, idx_maps = _prepare(features, begins, ends)
    nc = _build_program(ranges, fills, nkt)
    res = run_bass_kernel_spmd(nc, in_maps, list(range(B)), trace=trace)
    out = np.zeros((B, W, D), np.float32)
    for c in range(B):
        idx = idx_maps[c]
        used = idx >= 0
        dev = res.results[c]["out"].astype(np.float32)
        out[c, idx[used]] = dev[used]
    return out, res


def kernel(features, begins, ends):
    out, _ = run(features, begins, ends, trace=False)
    return out
